# revision 1
# baseline (speedup 1.0000x reference)
"""LOCA-style kernel for Trainium2, data-parallel over batch on 8 NeuronCores.

Per core (one batch element):
  - 3x3 depthwise correlation for 3 objects x 256 ch: diagonal-weight PE
    matmuls accumulating 9 taps in PSUM (+ optional DVE scalar_tensor_tensor
    taps for engine balancing)
  - softmax over the 3 objects + softmax-weighted sum: exp on ScalarE,
    elementwise on DVE/GPSIMD, 1/x computed as exp(-ln(x)) so every ACT
    transcendental stays in one table set
  - 1x1 head projection: PE matmuls with sign(w_head) stationary, |w_head|
    folded into the ln() scale
  - ReLU + 8x bilinear upsample: two separable PE matmul passes with a
    host-precomputed interpolation matrix
All on-chip data is fp16 (fp32 PSUM accumulation); output is fp32.
"""

import sys

sys.path.insert(0, "/opt/trn_rl_repo")

import numpy as np
from contextlib import ExitStack

import concourse.bass as bass
import concourse.mybir as mybir
from concourse import bacc, tile
from concourse.bass_utils import run_bass_kernel_spmd

BS, C, H, W = 8, 256, 64, 64
STEPS, NO = 3, 3
RED = 8
HO, WO = H * RED, W * RED  # 512, 512
HP, WP = H + 2, W + 4  # padded rows=66, padded row stride=68 (even)
NCORES = 8
NCT = 2  # channel tiles of 128
HW = H * W  # 4096
GR = 1024  # conv psum granule (free dim elements)
NGR = HW // GR
F16 = mybir.dt.float16
F32 = mybir.dt.float32
AF = mybir.ActivationFunctionType
ALU = mybir.AluOpType

# Engine split for the 9 conv taps of each object (tap = (ki+1)*3 + (kj+1)).
# Taps in PE_TAPS run as diag-matmuls into PSUM; taps in DVE_TAPS run as
# scalar_tensor_tensor FMAs on top of the evacuated SBUF tile.
PE_TAPS = {o: [0, 2, 3, 4, 5, 6, 8] for o in range(NO)}
DVE_TAPS = {o: [1, 7] for o in range(NO)}
# how many granule evacuations per (ct, step) go to DVE (rest to ScalarE)
DVE_EVACS = 0


def _bilinear_matrix(n_in: int, n_out: int) -> np.ndarray:
    """Matrix of jax.image.resize(method="bilinear") upsampling:
    half-pixel centers, edge clamp."""
    U = np.zeros((n_out, n_in), np.float64)
    s = n_in / n_out
    for i in range(n_out):
        c = (i + 0.5) * s - 0.5
        lo = int(np.floor(c))
        f = c - lo
        for idx, wt in ((lo, 1.0 - f), (lo + 1, f)):
            U[i, min(max(idx, 0), n_in - 1)] += wt
    return U


def _host_prep(f_e, all_prototypes, w_head, b_head):
    f_e = np.asarray(f_e, np.float32)
    ap = np.asarray(all_prototypes, np.float32)
    w_head = np.asarray(w_head, np.float32)
    b_val = float(np.asarray(b_head).reshape(-1)[0])

    # padded feature map; alignment A: x lives at cols 1..64, B: cols 0..63
    fpad = np.zeros((BS, 2, NCT, 128, HP, WP), np.float16)
    for ct in range(NCT):
        blk = f_e[:, ct * 128 : (ct + 1) * 128]  # [BS,128,H,W]
        fpad[:, 0, ct, :, 1 : 1 + H, 1 : 1 + W] = blk
        fpad[:, 1, ct, :, 1 : 1 + H, 0:W] = blk
    fpad = fpad.reshape(BS, 2, NCT, 128, HP * WP)

    # all_prototypes[s, j=o*9+tap, b, c]; diag[b, s, ct, c, j, c'] = w iff c'==c
    w = ap.astype(np.float16)  # [STEPS, 27, BS, C]
    diag = np.zeros((BS, STEPS, NCT, 128, NO * 9, 128), np.float16)
    cidx = np.arange(128)
    for ct in range(NCT):
        wct = w[:, :, :, ct * 128 : (ct + 1) * 128]  # [S, 27, BS, 128]
        # advanced idx on axes (3,5) -> result [128, BS, S, 27]
        diag[:, :, ct, cidx, :, cidx] = wct.transpose(3, 2, 0, 1)

    # per-partition tap weights for the DVE path: [BS, NCT, 128, S*27]
    wcolt = np.zeros((BS, NCT, 128, STEPS * 27), np.float16)
    wtr = w.transpose(2, 3, 0, 1).reshape(BS, C, STEPS * 27)  # [BS, C, S*27]
    for ct in range(NCT):
        wcolt[:, ct] = wtr[:, ct * 128 : (ct + 1) * 128]

    absw = np.abs(w_head).astype(np.float64)
    invw = np.where(absw > 0, 1.0 / np.maximum(absw, 1e-38), 3.0e38)
    invw = np.minimum(invw, 3.0e38).astype(np.float32)
    signw = np.sign(w_head).astype(np.float16)
    invw_t = np.ascontiguousarray(invw.reshape(NCT, 128, 1))
    signw_t = np.ascontiguousarray(signw.reshape(NCT, 128, 1))

    ut = _bilinear_matrix(H, HO).T.astype(np.float16)  # [64, 512]
    eye = np.eye(128, dtype=np.float16)

    in_maps = []
    for b in range(BS):
        in_maps.append(
            {
                "fpad": np.ascontiguousarray(fpad[b]),
                "diag": np.ascontiguousarray(diag[b]),
                "wcol": np.ascontiguousarray(wcolt[b]),
                "invw": invw_t,
                "signw": signw_t,
                "ut": ut,
                "eye": eye,
            }
        )
    return in_maps, b_val


def _build_nc(b_val: float) -> bass.Bass:
    nc = bacc.Bacc(None, target_bir_lowering=False)
    fpad_d = nc.declare_dram_parameter("fpad", [2, NCT, 128, HP * WP], F16, isOutput=False)
    diag_d = nc.declare_dram_parameter("diag", [STEPS, NCT, 128, NO * 9, 128], F16, isOutput=False)
    wcol_d = nc.declare_dram_parameter("wcol", [NCT, 128, STEPS * 27], F16, isOutput=False)
    invw_d = nc.declare_dram_parameter("invw", [NCT, 128, 1], F32, isOutput=False)
    signw_d = nc.declare_dram_parameter("signw", [NCT, 128, 1], F16, isOutput=False)
    ut_d = nc.declare_dram_parameter("ut", [64, WO], F16, isOutput=False)
    eye_d = nc.declare_dram_parameter("eye", [128, 128], F16, isOutput=False)
    out_d = nc.declare_dram_parameter("out", [STEPS, HO, WO], F32, isOutput=True)

    with tile.TileContext(nc) as tc, ExitStack() as ctx:
        const = ctx.enter_context(tc.tile_pool(name="const", bufs=1))
        fpool = ctx.enter_context(tc.tile_pool(name="fpool", bufs=1))
        dpool = ctx.enter_context(tc.tile_pool(name="dpool", bufs=2))
        spool = ctx.enter_context(tc.tile_pool(name="spool", bufs=2))
        mpool = ctx.enter_context(tc.tile_pool(name="mpool", bufs=1))
        rpool = ctx.enter_context(tc.tile_pool(name="rpool", bufs=2))
        opool = ctx.enter_context(tc.tile_pool(name="opool", bufs=2))
        ps_conv = ctx.enter_context(tc.tile_pool(name="ps_conv", bufs=2, space="PSUM"))
        ps_pd = ctx.enter_context(tc.tile_pool(name="ps_pd", bufs=2, space="PSUM"))
        ps_ups = ctx.enter_context(tc.tile_pool(name="ps_ups", bufs=2, space="PSUM"))

        # ---- constants ----
        ut_sb = const.tile([64, WO], F16, tag="ut")
        nc.sync.dma_start(out=ut_sb[:], in_=ut_d[:])
        eye_sb = const.tile([128, 128], F16, tag="eye")
        nc.sync.dma_start(out=eye_sb[:], in_=eye_d[:])
        invw_sb, signw_sb, wcol_sb = [], [], []
        for ct in range(NCT):
            t = const.tile([128, 1], F32, tag=f"invw{ct}")
            nc.sync.dma_start(out=t[:], in_=invw_d[ct])
            invw_sb.append(t)
            t = const.tile([128, 1], F16, tag=f"signw{ct}")
            nc.sync.dma_start(out=t[:], in_=signw_d[ct])
            signw_sb.append(t)
            t = const.tile([128, STEPS * 27], F16, tag=f"wcol{ct}")
            nc.sync.dma_start(out=t[:], in_=wcol_d[ct])
            wcol_sb.append(t)

        # ---- feature map (padded, two alignments) ----
        fsb = {}
        for al in range(2):
            for ct in range(NCT):
                t = fpool.tile([128, HP * WP], F16, tag=f"f{al}{ct}")
                nc.sync.dma_start(out=t[:], in_=fpad_d[al, ct])
                fsb[(al, ct)] = t

        def f_ap(ct, tap, row0, nrow):
            """Shifted padded-f AP covering output rows row0..row0+nrow-1."""
            ki, kj = tap // 3 - 1, tap % 3 - 1
            if kj == 0:
                al, col0 = 1, 0  # B alignment: x at col 0 (even offset)
            else:
                al, col0 = 0, 1 + kj  # A alignment: x at col 1
            v = fsb[(al, ct)][:].rearrange("p (r c) -> p r c", c=WP)
            return v[:, row0 + 1 + ki : row0 + 1 + ki + nrow, col0 : col0 + W]

        for s in range(STEPS):
            red_tiles = {}
            for ct in range(NCT):
                dg = dpool.tile([128, NO * 9 * 128], F16, tag="diag")
                nc.sync.dma_start(out=dg[:], in_=diag_d[s, ct])
                dgv = dg[:].rearrange("p (j c) -> p j c", c=128)

                resp = spool.tile([128, NO * HW], F16, tag="resp")
                respv = resp[:].rearrange("p (o q) -> p o q", o=NO)

                # ---- depthwise conv ----
                nevac = 0
                for o in range(NO):
                    pe_taps = PE_TAPS[o]
                    assert pe_taps, "each object needs at least one PE tap"
                    for g in range(NGR):
                        acc = ps_conv.tile([128, GR], F32, tag="acc")
                        nrow = GR // W
                        for ti, tap in enumerate(pe_taps):
                            for sub in range(GR // 512):
                                nc.tensor.matmul(
                                    acc[:, sub * 512 : (sub + 1) * 512],
                                    dgv[:, o * 9 + tap, :],
                                    f_ap(ct, tap, g * nrow + sub * 8, 8),
                                    start=(ti == 0),
                                    stop=(ti == len(pe_taps) - 1),
                                )
                        dst = respv[:, o, g * GR : (g + 1) * GR]
                        if nevac < DVE_EVACS:
                            nc.vector.tensor_copy(dst, acc[:])
                        else:
                            nc.scalar.activation(dst, acc[:], AF.Copy)
                        nevac += 1
                    for tap in DVE_TAPS[o]:
                        wix = s * 27 + o * 9 + tap
                        nc.vector.scalar_tensor_tensor(
                            respv[:, o, :],
                            f_ap(ct, tap, 0, H),
                            wcol_sb[ct][:, wix : wix + 1],
                            respv[:, o, :],
                            op0=ALU.mult,
                            op1=ALU.add,
                        )

                # ---- softmax over objects + weighted sum ----
                e = mpool.tile([128, NO * HW], F16, tag="e")
                nc.scalar.activation(e[:], resp[:], AF.Exp)
                ev = e[:].rearrange("p (o q) -> p o q", o=NO)

                den = mpool.tile([128, HW], F16, tag="den")
                nc.gpsimd.tensor_add(den[:], ev[:, 0, :], ev[:, 1, :])
                nc.gpsimd.tensor_add(den[:], den[:], ev[:, 2, :])

                er = mpool.tile([128, NO * HW], F16, tag="er")
                nc.vector.tensor_mul(er[:], e[:], resp[:])
                erv = er[:].rearrange("p (o q) -> p o q", o=NO)
                num = mpool.tile([128, HW], F16, tag="num")
                nc.vector.tensor_add(num[:], erv[:, 0, :], erv[:, 1, :])
                nc.vector.tensor_add(num[:], num[:], erv[:, 2, :])

                # recipw = |w|/den via exp(-ln(den/|w|))
                rw = mpool.tile([128, HW], F16, tag="rw")
                nc.scalar.activation(rw[:], den[:], AF.Ln, scale=invw_sb[ct][:])
                nc.scalar.activation(rw[:], rw[:], AF.Exp, scale=-1.0)

                red = rpool.tile([128, HW], F16, tag="red")
                nc.gpsimd.tensor_mul(red[:], num[:], rw[:])
                red_tiles[ct] = red

            # ---- head: dmap[1, yx] = sum_c sign(w)*red, chunked by PSUM bank ----
            dmY = opool.tile([64, 64], F16, tag="dmY")
            for k in range(HW // 512):
                pd = ps_pd.tile([1, 512], F32, tag="pd")
                for ct in range(NCT):
                    nc.tensor.matmul(
                        pd[:],
                        signw_sb[ct][:],
                        red_tiles[ct][:, k * 512 : (k + 1) * 512],
                        start=(ct == 0),
                        stop=(ct == NCT - 1),
                    )
                # relu(x + b) on the way out
                dm1 = opool.tile([1, 512], F16, tag="dm1")
                nc.vector.tensor_scalar(
                    dm1[:],
                    pd[:],
                    b_val,
                    0.0,
                    op0=ALU.add,
                    op1=ALU.max,
                )
                # scatter [1, 512] -> rows 8k..8k+7 of dmY (y on partitions)
                nc.sync.dma_start(
                    out=dmY[8 * k : 8 * k + 8, :],
                    in_=dm1[:].rearrange("p (y x) -> p y x", x=64),
                )
            # transpose to put x on partitions
            psT0 = ps_ups.tile([64, 64], F16, tag="upst")
            nc.tensor.transpose(psT0[:], dmY[:], eye_sb[0:64, 0:64])
            dmX = opool.tile([64, 64], F16, tag="dmX")
            nc.vector.tensor_copy(dmX[:], psT0[:])

            # horizontal upsample: G[X, y] = sum_x ut[x, X] * dmX[x, y]
            ps_h = ps_ups.tile([128, 512], F32, tag="upst")
            for xc in range(4):
                nc.tensor.matmul(
                    ps_h[:, xc * 64 : (xc + 1) * 64],
                    ut_sb[:, xc * 128 : (xc + 1) * 128],
                    dmX[:],
                    start=True,
                    stop=True,
                )
            h_sb = opool.tile([128, 256], F16, tag="h_sb")
            nc.scalar.activation(h_sb[:], ps_h[:, 0:256], AF.Copy)
            hyT = opool.tile([64, 512], F16, tag="hyT")
            for xc in range(4):
                psTx = ps_ups.tile([64, 128], F16, tag="upst")
                nc.tensor.transpose(
                    psTx[:],
                    h_sb[:, xc * 64 : (xc + 1) * 64],
                    eye_sb[:],
                )
                nc.vector.tensor_copy(
                    hyT[:, xc * 128 : (xc + 1) * 128], psTx[:]
                )
            # vertical upsample: out[Y, X] = sum_y ut[y, Y] * hyT[y, X]
            for yc in range(4):
                pv = ps_ups.tile([128, 512], F32, tag="upst")
                nc.tensor.matmul(
                    pv[:],
                    ut_sb[:, yc * 128 : (yc + 1) * 128],
                    hyT[:],
                    start=True,
                    stop=True,
                )
                osb = opool.tile([128, 512], F32, tag="osb")
                nc.vector.tensor_copy(osb[:], pv[:])
                nc.sync.dma_start(
                    out=out_d[s, yc * 128 : (yc + 1) * 128, :], in_=osb[:]
                )

    nc.compile()
    return nc


_CACHE = {}


def _get_nc(b_val: float) -> bass.Bass:
    key = round(b_val, 12)
    if key not in _CACHE:
        _CACHE[key] = _build_nc(b_val)
    return _CACHE[key]


def kernel(f_e, all_prototypes, w_head, b_head):
    in_maps, b_val = _host_prep(f_e, all_prototypes, w_head, b_head)
    nc = _get_nc(b_val)
    res = run_bass_kernel_spmd(nc, in_maps, list(range(NCORES)), trace=False)
    outs = [res.results[b]["out"].reshape(STEPS, 1, HO, WO) for b in range(BS)]
    full = np.stack(outs, axis=1)  # [STEPS, BS, 1, HO, WO]
    return full.astype(np.float32)

